# revision 25
# baseline (speedup 1.0000x reference)
"""MHA kernel for trn2: B=4, T=2048, D=2048, NH=16, HD=128, causal, no scale.

Sharding: 8 cores = 4 batches x 2 head-groups (8 heads each core).
Fully fused per-head pipeline per core:
  for h: QKV^T projection (fp32r matmuls, x^T resident in SBUF) writes
  K^T/Q^T straight into SBUF; V^T is PE-transposed to V (s-major); then
  causal attention in K-major layout: S^T = K^T.T @ Q^T, E = exp(S^T)
  (no max subtraction - logits are provably small), diagonal chunks are
  width-truncated + masked, O^T_unnorm = sum_s V[s].T @ E[s] in PSUM,
  l = ones.T @ E accumulated in PSUM. Normalization happens on host.
"""
import sys

sys.path.insert(0, '/opt/trn_rl_repo')

import numpy as np
import concourse.bass as bass
import concourse.mybir as mybir
import concourse.tile as tile
from concourse import bacc, bass_utils
from concourse.masks import make_identity

B, T, D = 4, 2048, 2048
NH, HD = 16, 128
HG = 2                      # head groups (tensor-parallel dim)
H_PER = NH // HG            # 8 heads per core
M_CHUNKS = 3 * H_PER        # 24 (q,k,v) x heads, 128 rows each
KO = D // 128               # 16 contraction chunks
TT = T // 512               # 4 t-tiles
SC = T // 128               # 16 s-chunks

f32 = mybir.dt.float32
f32r = mybir.dt.float32r

# diagonal chunk k (s0 = t0 + 128k): compute columns [j0, 512) of the t-tile
DIAG_W = [512, 384, 256, 256]
DIAG_J0 = [0, 128, 256, 256]
DIAG_MOFF = [0, 0, 0, 512]   # mask slice offset into mk_sb [m0(512) | m1(256)]

_REPEAT = 1


def round_fp32r(x):
    u = np.ascontiguousarray(x, dtype=np.float32).view(np.uint32).copy()
    u += 0x7FF + ((u >> 12) & 1)
    u &= np.uint32(0xFFFFF000)
    return u.view(np.float32)


def build_nc(repeat=1, bench_mode=False):
    nc = bacc.Bacc("TRN2", target_bir_lowering=False, debug=False)
    if not bench_mode:
        xt_d = nc.dram_tensor("xt", [128, KO, T], f32r, kind="ExternalInput")
        wt_d = nc.dram_tensor("wt", [M_CHUNKS, 128, KO, 128], f32r,
                              kind="ExternalInput")
        mk_d = nc.dram_tensor("mk", [128, 768], f32r, kind="ExternalInput")
    o_d = nc.dram_tensor("o_un", [H_PER, 128, T], f32, kind="ExternalOutput")
    l_d = nc.dram_tensor("l_acc", [H_PER, T], f32, kind="ExternalOutput")

    if bench_mode:
        # timing-only: inputs live in internal DRAM (garbage data), so
        # repeated executions ship no host data
        xt_d = nc.dram_tensor("xt", [128, KO, T], f32r, kind="Internal")
        wt_d = nc.dram_tensor("wt", [M_CHUNKS, 128, KO, 128], f32r,
                              kind="Internal")
        mk_d = nc.dram_tensor("mk", [128, 768], f32r, kind="Internal")

    with tile.TileContext(nc) as tc:
        with tc.tile_pool(name="const", bufs=1) as cpool:
            ident = cpool.tile([128, 128], f32)
            make_identity(nc, ident[:])
            ident_r = cpool.tile([128, 128], f32r)
            nc.vector.tensor_copy(ident_r[:], ident[:])
            mk_sb = cpool.tile([128, 768], f32r)
            nc.sync.dma_start(mk_sb[:], mk_d.ap())
            ones_f = cpool.tile([128, 8], f32)
            nc.gpsimd.memset(ones_f[:], 1.0)
            ones_sb = cpool.tile([128, 8], f32r)
            nc.vector.tensor_copy(ones_sb[:], ones_f[:])

            for _ in range(repeat):
                with tc.tile_pool(name="xsb", bufs=1) as xpool, \
                     tc.tile_pool(name="wsb", bufs=2) as wpool, \
                     tc.tile_pool(name="kv", bufs=2) as kvpool, \
                     tc.tile_pool(name="qt", bufs=3) as qpool, \
                     tc.tile_pool(name="vts", bufs=2) as vtpool, \
                     tc.tile_pool(name="esb", bufs=3) as epool, \
                     tc.tile_pool(name="etm", bufs=2) as etpool, \
                     tc.tile_pool(name="lsb", bufs=2) as lpool, \
                     tc.tile_pool(name="osb", bufs=1) as opool, \
                     tc.tile_pool(name="pps", bufs=2, space="PSUM") as ppool, \
                     tc.tile_pool(name="sps", bufs=3, space="PSUM") as sps, \
                     tc.tile_pool(name="ops", bufs=1, space="PSUM") as ops, \
                     tc.tile_pool(name="lps", bufs=1, space="PSUM") as lps, \
                     tc.tile_pool(name="tps", bufs=1, space="PSUM") as tps:
                    x_sb = xpool.tile([128, KO, T], f32r)
                    nc.sync.dma_start(x_sb[:], xt_d.ap())

                    for h in range(H_PER):
                        k_sb = kvpool.tile([128, T], f32r, tag="k")
                        v_sb = kvpool.tile([128, SC, 128], f32r, tag="v")
                        q_ts = []

                        # ---- projection for head h: c=0 (q), 1 (k), 2 (v) ----
                        for c in range(3):
                            m = c * H_PER + h
                            w_sb = wpool.tile([128, KO, 128], f32r, tag="w")
                            nc.sync.dma_start(w_sb[:], wt_d.ap()[m])
                            for t in range(TT):
                                pt = ppool.tile([128, 512], f32, tag="p")
                                for ko in range(KO):
                                    nc.tensor.matmul(
                                        pt[:], w_sb[:, ko],
                                        x_sb[:, ko, t * 512:(t + 1) * 512],
                                        start=(ko == 0), stop=(ko == KO - 1))
                                if c == 0:
                                    q_t = qpool.tile([128, 512], f32r, tag="q")
                                    nc.vector.tensor_copy(q_t[:], pt[:])
                                    q_ts.append(q_t)
                                elif c == 1:
                                    nc.vector.tensor_copy(
                                        k_sb[:, t * 512:(t + 1) * 512], pt[:])
                                else:
                                    vt_st = vtpool.tile([128, 512], f32r, tag="vt")
                                    nc.vector.tensor_copy(vt_st[:], pt[:])
                                    for j in range(4):
                                        s = 4 * t + j
                                        tp = tps.tile([128, 128], f32r, tag="tp")
                                        nc.tensor.transpose(
                                            tp[:], vt_st[:, j * 128:(j + 1) * 128],
                                            ident_r[:])
                                        nc.vector.tensor_copy(v_sb[:, s], tp[:])

                        # ---- attention for head h ----
                        for t in range(TT):
                            t0 = t * 512
                            n_chunks = 4 * (t + 1)
                            op = ops.tile([128, 512], f32, tag="op")
                            lp = lps.tile([1, 512], f32, tag="lp")
                            e_info = []

                            def emit_pv(idx):
                                et_, j0_, w_ = e_info[idx]
                                s_ = idx
                                nc.tensor.matmul(
                                    op[:, j0_:j0_ + w_], v_sb[:, s_], et_[:, 0:w_],
                                    start=(s_ == 0), stop=(s_ == n_chunks - 1))
                                nc.tensor.matmul(
                                    lp[:, j0_:j0_ + w_], ones_sb[:, 0:1], et_[:, 0:w_],
                                    start=(s_ == 0), stop=(s_ == n_chunks - 1))

                            for s in range(n_chunks):
                                k_diag = s - 4 * t
                                if k_diag >= 0:
                                    w, j0, moff = (DIAG_W[k_diag], DIAG_J0[k_diag],
                                                   DIAG_MOFF[k_diag])
                                else:
                                    w, j0 = 512, 0
                                sp = sps.tile([128, 512], f32, tag="sp")
                                nc.tensor.matmul(
                                    sp[:, 0:w], k_sb[:, s * 128:(s + 1) * 128],
                                    q_ts[t][:, j0:j0 + w], start=True, stop=True)
                                et = epool.tile([128, 512], f32r, tag="e")
                                if k_diag >= 0:
                                    etmp = etpool.tile([128, 512], f32r, tag="etmp")
                                    nc.scalar.activation(
                                        etmp[:, 0:w], sp[:, 0:w],
                                        mybir.ActivationFunctionType.Exp)
                                    nc.vector.tensor_tensor(
                                        et[:, 0:w], etmp[:, 0:w],
                                        mk_sb[:, moff:moff + w], mybir.AluOpType.mult)
                                else:
                                    nc.scalar.activation(
                                        et[:, 0:w], sp[:, 0:w],
                                        mybir.ActivationFunctionType.Exp)
                                e_info.append((et, j0, w))
                                if s >= 1:
                                    emit_pv(s - 1)
                            emit_pv(n_chunks - 1)

                            l_sb = lpool.tile([1, 512], f32, tag="l")
                            nc.vector.tensor_copy(l_sb[:], lp[:])
                            o_sb = opool.tile([128, 512], f32, tag="o")
                            nc.vector.tensor_copy(o_sb[:], op[:])
                            nc.sync.dma_start(o_d.ap()[h, :, t0:t0 + 512], o_sb[:])
                            nc.sync.dma_start(
                                l_d.ap()[h, t0:t0 + 512].unsqueeze(0), l_sb[:])
    nc.compile()
    return nc


def _host_prep(x, qkv_proj):
    """Build per-core input maps. Cores: c -> (b = c // 2, hg = c % 2)."""
    xts = []
    for b in range(B):
        xt = round_fp32r(x[b].T)                      # [D, T]
        xts.append(np.ascontiguousarray(
            xt.reshape(KO, 128, T).transpose(1, 0, 2)))
    wts = []
    for hg in range(HG):
        w = qkv_proj[:, hg * (H_PER * HD):(hg + 1) * (H_PER * HD), :]
        w = w.reshape(3 * H_PER * HD, D)              # [3072, D]
        wt = round_fp32r(w.T)                         # [D, 3072]
        wts.append(np.ascontiguousarray(
            wt.reshape(KO, 128, M_CHUNKS, 128).transpose(2, 1, 0, 3)))
    # masks: mk = [m0 (512 cols) | m1 (256 cols)]
    # m0[i, j] = (i <= j); m1[i, j] = (i + 128 <= j)
    mk = np.zeros((128, 768), np.float32)
    ii = np.arange(128)[:, None]
    mk[:, 0:512] = (ii <= np.arange(512)[None, :]).astype(np.float32)
    mk[:, 512:768] = (ii + 128 <= np.arange(256)[None, :]).astype(np.float32)
    in_maps = []
    for c in range(8):
        b, hg = c // 2, c % 2
        in_maps.append({"xt": xts[b], "wt": wts[hg], "mk": mk})
    return in_maps


def _assemble(results):
    out = np.empty((B, T, NH * HD), np.float32)
    for c in range(8):
        b, hg = c // 2, c % 2
        o_un = results[c]["o_un"].astype(np.float64)    # [H_PER, 128, T]
        l_sum = results[c]["l_acc"].astype(np.float64)  # [H_PER, T]
        o = o_un / l_sum[:, None, :]
        out[b, :, hg * (H_PER * HD):(hg + 1) * (H_PER * HD)] = (
            o.transpose(2, 0, 1).reshape(T, H_PER * HD))
    return out


_NC_CACHE = {}


def _get_nc(repeat=1):
    if repeat not in _NC_CACHE:
        _NC_CACHE[repeat] = build_nc(repeat)
    return _NC_CACHE[repeat]


def kernel(x, qkv_proj):
    x = np.asarray(x, np.float32)
    qkv_proj = np.asarray(qkv_proj, np.float32)
    nc = _get_nc(_REPEAT)
    in_maps = _host_prep(x, qkv_proj)
    res = bass_utils.run_bass_kernel_spmd(nc, in_maps, core_ids=list(range(8)))
    return _assemble(res.results)


# revision 30
# speedup vs baseline: 2.6307x; 2.6307x over previous
"""MHA kernel for trn2: B=4, T=2048, D=2048, NH=16, HD=128, causal, no scale.

Sharding: 8 cores = 4 batches x 2 head-groups (8 heads each core).
Fully fused per-head pipeline per core:
  for h: QKV^T projection (fp32r matmuls, x^T resident in SBUF) writes
  K^T/Q^T straight into SBUF; V^T is PE-transposed to V (s-major); then
  causal attention in K-major layout: S^T = K^T.T @ Q^T, E = exp(S^T)
  (no max subtraction - logits are provably small), diagonal chunks are
  width-truncated + masked, O^T_unnorm = sum_s V[s].T @ E[s] in PSUM,
  l = ones.T @ E accumulated in PSUM. Normalization happens on host.
"""
import sys

sys.path.insert(0, '/opt/trn_rl_repo')

import numpy as np
import concourse.bass as bass
import concourse.mybir as mybir
import concourse.tile as tile
from concourse import bacc, bass_utils
from concourse.masks import make_identity

B, T, D = 4, 2048, 2048
NH, HD = 16, 128
HG = 2                      # head groups (tensor-parallel dim)
H_PER = NH // HG            # 8 heads per core
M_CHUNKS = 3 * H_PER        # 24 (q,k,v) x heads, 128 rows each
KO = D // 128               # 16 contraction chunks
TT = T // 512               # 4 t-tiles
SC = T // 128               # 16 s-chunks

f32 = mybir.dt.float32
f32r = mybir.dt.float32r

# diagonal chunk k (s0 = t0 + 128k): compute columns [j0, 512) of the t-tile
DIAG_W = [512, 384, 256, 256]
DIAG_J0 = [0, 128, 256, 256]
DIAG_MOFF = [0, 0, 0, 512]   # mask slice offset into mk_sb [m0(512) | m1(256)]

_REPEAT = 1


def round_fp32r(x):
    u = np.ascontiguousarray(x, dtype=np.float32).view(np.uint32).copy()
    u += 0x7FF + ((u >> 12) & 1)
    u &= np.uint32(0xFFFFF000)
    return u.view(np.float32)


def build_nc(repeat=1, bench_mode=False):
    nc = bacc.Bacc("TRN2", target_bir_lowering=False, debug=False)
    if not bench_mode:
        xt_d = nc.dram_tensor("xt", [128, KO, T], f32r, kind="ExternalInput")
        wt_d = nc.dram_tensor("wt", [M_CHUNKS, 128, KO, 128], f32r,
                              kind="ExternalInput")
        mk_d = nc.dram_tensor("mk", [128, 768], f32r, kind="ExternalInput")
    o_d = nc.dram_tensor("o_un", [H_PER, 128, T], f32, kind="ExternalOutput")
    l_d = nc.dram_tensor("l_acc", [H_PER, 128, T], f32, kind="ExternalOutput")

    if bench_mode:
        # timing-only: inputs live in internal DRAM (garbage data), so
        # repeated executions ship no host data
        xt_d = nc.dram_tensor("xt", [128, KO, T], f32r, kind="Internal")
        wt_d = nc.dram_tensor("wt", [M_CHUNKS, 128, KO, 128], f32r,
                              kind="Internal")
        mk_d = nc.dram_tensor("mk", [128, 768], f32r, kind="Internal")

    with tile.TileContext(nc) as tc:
        with tc.tile_pool(name="const", bufs=1) as cpool:
            ident = cpool.tile([128, 128], f32)
            make_identity(nc, ident[:])
            ident_r = cpool.tile([128, 128], f32r)
            nc.vector.tensor_copy(ident_r[:], ident[:])
            mk_sb = cpool.tile([128, 768], f32r)
            nc.sync.dma_start(mk_sb[:], mk_d.ap())

            for _ in range(repeat):
                with tc.tile_pool(name="xsb", bufs=1) as xpool, \
                     tc.tile_pool(name="wsb", bufs=2) as wpool, \
                     tc.tile_pool(name="kv", bufs=2) as kvpool, \
                     tc.tile_pool(name="qt", bufs=3) as qpool, \
                     tc.tile_pool(name="vts", bufs=1) as vtpool, \
                     tc.tile_pool(name="esb", bufs=3) as epool, \
                     tc.tile_pool(name="etm", bufs=1) as etpool, \
                     tc.tile_pool(name="lsb", bufs=2) as lpool, \
                     tc.tile_pool(name="osb", bufs=1) as opool, \
                     tc.tile_pool(name="pps", bufs=2, space="PSUM") as ppool, \
                     tc.tile_pool(name="sps", bufs=3, space="PSUM") as sps, \
                     tc.tile_pool(name="ops", bufs=2, space="PSUM") as ops, \
                     tc.tile_pool(name="tps", bufs=1, space="PSUM") as tps:
                    x_sb = xpool.tile([128, KO, T], f32r)
                    nc.sync.dma_start(x_sb[:], xt_d.ap())

                    for h in range(H_PER):
                        k_sb = kvpool.tile([128, T], f32r, tag="k")
                        v_sb = kvpool.tile([128, SC, 128], f32r, tag="v")
                        q_ts = []

                        # ---- projection for head h: c=0 (q), 1 (k), 2 (v) ----
                        for c in range(3):
                            m = c * H_PER + h
                            w_sb = wpool.tile([128, KO, 128], f32r, tag="w")
                            nc.sync.dma_start(w_sb[:], wt_d.ap()[m])
                            for t in range(TT):
                                pt = ppool.tile([128, 512], f32, tag="p")
                                for ko in range(KO):
                                    nc.tensor.matmul(
                                        pt[:], w_sb[:, ko],
                                        x_sb[:, ko, t * 512:(t + 1) * 512],
                                        start=(ko == 0), stop=(ko == KO - 1))
                                if c == 0:
                                    q_t = qpool.tile([128, 512], f32r, tag="q")
                                    nc.vector.tensor_copy(q_t[:], pt[:])
                                    q_ts.append(q_t)
                                elif c == 1:
                                    nc.vector.tensor_copy(
                                        k_sb[:, t * 512:(t + 1) * 512], pt[:])
                                else:
                                    vt_st = vtpool.tile([128, 512], f32r, tag="vt")
                                    nc.vector.tensor_copy(vt_st[:], pt[:])
                                    for j in range(4):
                                        s = 4 * t + j
                                        tp = tps.tile([128, 128], f32r, tag="tp")
                                        nc.tensor.transpose(
                                            tp[:], vt_st[:, j * 128:(j + 1) * 128],
                                            ident_r[:])
                                        nc.vector.tensor_copy(v_sb[:, s], tp[:])

                        # ---- attention for head h ----
                        for t in range(TT):
                            t0 = t * 512
                            n_chunks = 4 * (t + 1)
                            op = ops.tile([128, 512], f32, tag="op")
                            l0 = lpool.tile([128, 512], f32, tag="l0")
                            l1 = lpool.tile([128, 512], f32, tag="l1")
                            e_info = []

                            def emit_pv(idx):
                                et_, j0_, w_ = e_info[idx]
                                s_ = idx
                                nc.tensor.matmul(
                                    op[:, j0_:j0_ + w_], v_sb[:, s_], et_[:, 0:w_],
                                    start=(s_ == 0), stop=(s_ == n_chunks - 1))

                            for s in range(n_chunks):
                                k_diag = s - 4 * t
                                if k_diag >= 0:
                                    w, j0, moff = (DIAG_W[k_diag], DIAG_J0[k_diag],
                                                   DIAG_MOFF[k_diag])
                                else:
                                    w, j0 = 512, 0
                                sp = sps.tile([128, 512], f32, tag="sp")
                                nc.tensor.matmul(
                                    sp[:, 0:w], k_sb[:, s * 128:(s + 1) * 128],
                                    q_ts[t][:, j0:j0 + w], start=True, stop=True)
                                et = epool.tile([128, 512], f32r, tag="e")
                                if k_diag >= 0:
                                    etmp = etpool.tile([128, 512], f32r, tag="etmp")
                                    nc.scalar.activation(
                                        etmp[:, 0:w], sp[:, 0:w],
                                        mybir.ActivationFunctionType.Exp)
                                    nc.vector.tensor_tensor(
                                        et[:, 0:w], etmp[:, 0:w],
                                        mk_sb[:, moff:moff + w], mybir.AluOpType.mult)
                                else:
                                    nc.scalar.activation(
                                        et[:, 0:w], sp[:, 0:w],
                                        mybir.ActivationFunctionType.Exp)
                                e_info.append((et, j0, w))
                                # l partial sums split across gpsimd and DVE
                                eng = nc.gpsimd if s % 2 else nc.vector
                                lx = l1 if s % 2 else l0
                                if s == 0:
                                    eng.tensor_copy(lx[:], et[:].bitcast(f32))
                                elif s == 1 and t > 0:
                                    eng.tensor_copy(lx[:], et[:].bitcast(f32))
                                else:
                                    if s == 1:  # t == 0: truncated first l1 chunk
                                        nc.gpsimd.memset(lx[:], 0.0)
                                    eng.tensor_tensor(
                                        lx[:, j0:j0 + w], lx[:, j0:j0 + w],
                                        et[:, 0:w].bitcast(f32),
                                        mybir.AluOpType.add)
                                if s >= 1:
                                    emit_pv(s - 1)
                            emit_pv(n_chunks - 1)

                            nc.vector.tensor_tensor(
                                l0[:], l0[:], l1[:], mybir.AluOpType.add)
                            o_sb = opool.tile([128, 512], f32, tag="o")
                            nc.vector.tensor_copy(o_sb[:], op[:])
                            nc.sync.dma_start(o_d.ap()[h, :, t0:t0 + 512], o_sb[:])
                            nc.sync.dma_start(l_d.ap()[h, :, t0:t0 + 512], l0[:])
    nc.compile()
    return nc


def _host_prep(x, qkv_proj):
    """Build per-core input maps. Cores: c -> (b = c // 2, hg = c % 2)."""
    xts = []
    for b in range(B):
        xt = round_fp32r(x[b].T)                      # [D, T]
        xts.append(np.ascontiguousarray(
            xt.reshape(KO, 128, T).transpose(1, 0, 2)))
    wts = []
    for hg in range(HG):
        w = qkv_proj[:, hg * (H_PER * HD):(hg + 1) * (H_PER * HD), :]
        w = w.reshape(3 * H_PER * HD, D)              # [3072, D]
        wt = round_fp32r(w.T)                         # [D, 3072]
        wts.append(np.ascontiguousarray(
            wt.reshape(KO, 128, M_CHUNKS, 128).transpose(2, 1, 0, 3)))
    # masks: mk = [m0 (512 cols) | m1 (256 cols)]
    # m0[i, j] = (i <= j); m1[i, j] = (i + 128 <= j)
    mk = np.zeros((128, 768), np.float32)
    ii = np.arange(128)[:, None]
    mk[:, 0:512] = (ii <= np.arange(512)[None, :]).astype(np.float32)
    mk[:, 512:768] = (ii + 128 <= np.arange(256)[None, :]).astype(np.float32)
    in_maps = []
    for c in range(8):
        b, hg = c // 2, c % 2
        in_maps.append({"xt": xts[b], "wt": wts[hg], "mk": mk})
    return in_maps


def _assemble(results):
    out = np.empty((B, T, NH * HD), np.float32)
    for c in range(8):
        b, hg = c // 2, c % 2
        o_un = results[c]["o_un"].astype(np.float64)       # [H_PER, 128, T]
        l_sum = results[c]["l_acc"].astype(np.float64).sum(axis=1)  # [H_PER, T]
        o = o_un / l_sum[:, None, :]
        out[b, :, hg * (H_PER * HD):(hg + 1) * (H_PER * HD)] = (
            o.transpose(2, 0, 1).reshape(T, H_PER * HD))
    return out


_NC_CACHE = {}


def _get_nc(repeat=1):
    if repeat not in _NC_CACHE:
        _NC_CACHE[repeat] = build_nc(repeat)
    return _NC_CACHE[repeat]


def kernel(x, qkv_proj):
    x = np.asarray(x, np.float32)
    qkv_proj = np.asarray(qkv_proj, np.float32)
    nc = _get_nc(_REPEAT)
    in_maps = _host_prep(x, qkv_proj)
    res = bass_utils.run_bass_kernel_spmd(nc, in_maps, core_ids=list(range(8)))
    return _assemble(res.results)
